# revision 1
# baseline (speedup 1.0000x reference)
"""Channel-attention (transposed attention) Trainium2 Bass kernel.

Reference computation (per batch b of 8, one NeuronCore each):
    X    = x[b].reshape(C, N).T                    # [N, C], N = 64*64 = 4096
    qkv  = X @ w_qkv                               # [N, 3C]
    q, k, v : per-head [N, hd], nh=8, hd=64
    logits_h = k_h.T @ v_h                         # [hd, hd]
    attn_h   = softmax(scale * logits_h, axis=-1)  # scale = hd**-0.5 = 1/8
    out_h    = q_h @ attn_h.T                      # [N, hd]
    y[b] = (concat_h(out_h) @ w_proj + b_proj).T   # [C, N]

Sharding: data-parallel over batch, 1 batch item per core, no collectives.

Algebraic restructuring (the whole point of this kernel):

1. Gram trick. logits_h = k_h^T v_h = Wk_h^T (X^T X) Wv_h, so k and v are
   never materialized. G = X^T X is one [C, C] matmul with contraction
   over the 4096 tokens (PE-transposes of x feed it), then
   T = G @ Wv  ([C, C]) and lg = Wk^T T (pair-packed [128,128] blocks)
   are tiny. This kills the [N, 2C] kv projection and its PSUM->SBUF
   copies entirely.

2. Weight folding. out_h = q_h @ (E_h / s_h)^T with E = exp(scale*(lg-max)),
   s = rowsum(E), and y^T = w_proj^T out^T + b. Fold:
       y^T = Wy^T X^T,  Wy = Wq @ M^T,  M^T = blockdiag(E_h)^T-applied
       via  M^T[64h+e, :] = sum_d E_h[d,e] * (w_proj[64h+d, :] / s_h[d])
   M^T is 4 [128,512] matmuls (block-diag exp as lhsT, row-scaled w_proj
   as rhs), Wy is 16 more. This kills the q^T projection ([C, N]) and
   the attention-apply stage: phase C is a single [C, C] x [C, N] GEMM
   reading the resident x tiles directly.

All large matmuls are float32r (fp32 bytes, FP22 multiply): 1 PE
cycle/column at free-dim >= 256 vs 4 for fp32. PE transposes in f32r
run 1.5 cycles/row vs 2.0 for fp32. The softmax itself (reductions,
exp) is exact fp32.

Per-rep PE budget ~210k cycles vs ~396k for the direct formulation.
"""

import numpy as np

import concourse.bass as bass
import concourse.mybir as mybir
import concourse.tile as tile
from concourse import bass_utils

F32 = mybir.dt.float32
F32R = mybir.dt.float32r
AF = mybir.ActivationFunctionType
AX = mybir.AxisListType.X

# Problem shape (hardcoded per contest contract).
B = 8
C = 512
H = W = 64
N = H * W            # 4096 tokens per batch
NH = 8               # heads
HD = C // NH         # 64
SCALE = HD ** -0.5   # 1/8
KC = C // 128        # 4 chunks of 128 channels
NS = 8               # n-slices of 512 tokens
SL = N // NS         # 512
TT = SL // 128       # 4 token tiles of 128 per slice
HP = NH // 2         # 4 head pairs


def _r(ap):
    return ap.bitcast(F32R)


def _f(ap):
    return ap.bitcast(F32)


def _split_multi_waits(nc, max_waits=1):
    """The walrus build in this container encodes at most one sync-wait
    command per instruction (setupSyncWait raises "Too many sync wait
    commands" otherwise — the Tile kernel-tail drain carries several).
    Hoist excess waits onto same-engine NOPs immediately preceding the
    instruction; engine-FIFO order preserves the semantics."""
    n_split = 0
    for bb in nc.main_func.blocks:
        new_insts = []
        for ins in bb.instructions:
            si = ins.sync_info
            waits = list(si.on_wait) if si and si.on_wait else []
            if len(waits) > max_waits:
                extra, keep = waits[:-max_waits], waits[-max_waits:]
                while extra:
                    chunk, extra = extra[:max_waits], extra[max_waits:]
                    nop = mybir.InstNoOp(
                        name=nc.get_next_instruction_name(),
                        ins=[], outs=[],
                        engine=ins.engine,
                        sync_info=mybir.SyncInfo(on_wait=chunk, on_update=[]),
                    )
                    nc.register_instruction(nop)
                    new_insts.append(nop)
                    n_split += 1
                si.on_wait = keep
            new_insts.append(ins)
        bb.instructions[:] = new_insts
    return n_split


def build_nc(reps=1, phases='full'):
    nc = bass.Bass("TRN2", debug=False, num_devices=B)

    x_t = nc.dram_tensor("x", [C, N], F32, kind="ExternalInput")
    wq_t = nc.dram_tensor("w_qkv", [C, 3 * C], F32, kind="ExternalInput")
    wp_t = nc.dram_tensor("w_proj", [C, C], F32, kind="ExternalInput")
    bp_t = nc.dram_tensor("b_proj", [C, 1], F32, kind="ExternalInput")
    y_t = nc.dram_tensor("y", [C, N], F32, kind="ExternalOutput")
    id_t = nc.inline_tensor(np.eye(128, dtype=np.float32), name="id128")

    xd, wqd, wpd, bpd, yd = x_t.ap(), wq_t.ap(), wp_t.ap(), bp_t.ap(), y_t.ap()

    with tile.TileContext(nc) as tc:
        with (
            tc.tile_pool(name="const", bufs=1) as cpool,
            tc.tile_pool(name="xres", bufs=1) as xrpool,
        ):
            # resident x: [C, N] as 4 chunks x 8 slices of [128, 512].
            # slice-0 chunks lead the DMA queue (opening transposes need
            # them), weights follow, slices 1..7 stream per-slice.
            x_sb = [
                [xrpool.tile([128, SL], F32R, name=f"x{k}_{ns}", tag=f"x{k}_{ns}")
                 for ns in range(NS)]
                for k in range(KC)
            ]
            id_sb = cpool.tile([128, 128], F32R, tag="id")
            # id leads the scalar DGE queue (tiny, needed by the first
            # transpose); x chunks interleave across BOTH DGE queues
            # ahead of the weights so the slice cadence (~2.8us) beats
            # phase A's consumption rate (~3.2us/slice).
            nc.scalar.dma_start(id_sb[:], _r(id_t.ap()[:, :]))
            for ns in range(NS):
                nsl = slice(ns * SL, (ns + 1) * SL)
                for k in range(KC):
                    eng = nc.sync if (ns * KC + k) % 2 == 0 else nc.scalar
                    eng.dma_start(x_sb[k][ns][:],
                                  _r(xd[k * 128:(k + 1) * 128, nsl]))

            wq_sb = [cpool.tile([128, 3 * C], F32R, name=f"wq{k}", tag=f"wq{k}")
                     for k in range(KC)]
            # section order k, v, q: k/v feed T+lg mid-phase-B, q only
            # feeds the Wq^T transposes emitted late in phase B
            for s in (1, 2, 0):
                cs = slice(s * C, (s + 1) * C)
                for k in range(KC):
                    r = slice(k * 128, (k + 1) * 128)
                    eng = nc.sync if k % 2 == 0 else nc.scalar
                    eng.dma_start(wq_sb[k][:, cs], _r(wqd[r, cs]))

            wp_sb = [cpool.tile([128, C], F32R, name=f"wp{k}", tag=f"wp{k}")
                     for k in range(KC)]
            bp_sb = [cpool.tile([128, 1], F32, name=f"bp{k}", tag=f"bp{k}")
                     for k in range(KC)]

            # Explicit PSUM choreography: 8 named banks, assigned so that
            # phase C (banks 6,7) of rep r is disjoint from phase A
            # (banks 0-5) of rep r+1 — the cross-rep overlap that hides
            # the next rep's transpose+Gram work under the projection.
            with tc.tile_pool(name="ps", bufs=1, space="PSUM") as pspool:
                bank = [pspool.tile([128, C], F32, name=f"bank{i}", tag=f"bank{i}")
                        for i in range(8)]

                wqT_sb = [cpool.tile([128, C], F32R, name=f"wqT{kq}", tag=f"wqT{kq}")
                          for kq in range(KC)]

                for _rep in range(reps):
                    _build_one_pass(nc, tc, cpool, wq_sb, wqT_sb, wp_sb, bp_sb,
                                    id_sb, x_sb, xd, yd, wpd, bpd, bank,
                                    first_rep=(_rep == 0), phases=phases)
    _split_multi_waits(nc)
    return nc


def _build_one_pass(nc, tc, cpool, wq_sb, wqT_sb, wp_sb, bp_sb, id_sb, x_sb,
                    xd, yd, wpd, bpd, bank, first_rep=True, phases="full"):
    lvl = ["dma", "gram", "logits", "soft", "wy", "full"].index(phases)

    # per-rep work tiles (stable tags: same storage every rep)
    G_sb = [cpool.tile([128, C], F32R, name=f"G{k}", tag=f"G{k}") for k in range(KC)]
    T_sb = [cpool.tile([128, C], F32R, name=f"T{k}", tag=f"T{k}") for k in range(KC)]
    mt_sb = [cpool.tile([128, C], F32R, name=f"mt{p}", tag=f"mt{p}") for p in range(HP)]
    wy_sb = [cpool.tile([128, C], F32R, name=f"wy{k}", tag=f"wy{k}") for k in range(KC)]
    bd = [cpool.tile([128, 128], F32, name=f"bd{p}", tag=f"bd{p}") for p in range(HP)]
    bd2 = [cpool.tile([128, 128], F32R, name=f"bd2{p}", tag=f"bd2{p}") for p in range(HP)]
    mx = cpool.tile([128, HP], F32, name="mx", tag="mx")
    bias = cpool.tile([128, HP], F32, name="bias", tag="bias")
    ssum = cpool.tile([128, HP], F32, name="ssum", tag="ssum")
    recip = cpool.tile([128, HP], F32, name="recip", tag="recip")

    # ================= Phase A: transpose x + Gram accumulation =========
    # banks 0-3: G accumulators; banks 4,5: transpose staging. Disjoint
    # from the previous rep's phase C (banks 6,7), so this phase runs
    # under it. x loads go on the Activation DGE queue so they are not
    # stuck behind the previous rep's y stores on the sync queue.
    G_ps = bank[0:KC]
    with tc.tile_pool(name="xt", bufs=4) as xtpool:
        for ns in range(NS):
            if not first_rep:
                # reloads for later passes ride the Activation DGE queue,
                # clear of the previous pass's y stores on the sync queue
                nsl = slice(ns * SL, (ns + 1) * SL)
                for k in range(KC):
                    nc.scalar.dma_start(
                        x_sb[k][ns][:], _r(xd[k * 128:(k + 1) * 128, nsl])
                    )
            if lvl < 1:
                continue
            for t in range(TT):
                tsl = slice(t * 128, (t + 1) * 128)
                tp = bank[4 + t % 2]
                for k in range(KC):
                    nc.tensor.matmul(
                        _r(tp[:, k * 128:(k + 1) * 128]),
                        x_sb[k][ns][:, tsl], id_sb[:], is_transpose=True,
                        start=(k == 0), stop=(k == KC - 1),
                    )
                xt = xtpool.tile([128, C], F32R, tag="xt_sb")
                # halve the PSUM->SBUF latency: each copy runs as two
                # column halves on DVE and ACT concurrently
                nc.vector.tensor_copy(xt[:, 0:C // 2], tp[:, 0:C // 2])
                nc.scalar.activation(xt[:, C // 2:C], tp[:, C // 2:C], AF.Copy)
                for k in range(KC):
                    nc.tensor.matmul(
                        G_ps[k][:], xt[:, k * 128:(k + 1) * 128], xt[:],
                        start=(ns == 0 and t == 0),
                        stop=(ns == NS - 1 and t == TT - 1),
                    )
        if lvl >= 1:
            for k in range(KC):
                nc.vector.tensor_copy(G_sb[k][:, 0:C // 2], G_ps[k][:, 0:C // 2])
                nc.scalar.activation(G_sb[k][:, C // 2:C], G_ps[k][:, C // 2:C],
                                     AF.Copy)

    # deferred weight loads: w_proj/b_proj first needed in phase B tail
    if first_rep:
        for k in range(KC):
            r = slice(k * 128, (k + 1) * 128)
            nc.sync.dma_start(wp_sb[k][:], _r(wpd[r, :]))
            nc.sync.dma_start(bp_sb[k][:], bpd[r, :])

    if lvl < 2:
        return
    # ================= Phase B: T = G Wv, logits, softmax, M^T, Wy ======
    for kc in range(KC):
        Tp = bank[6 + kc % 2]
        for k2 in range(KC):
            nc.tensor.matmul(
                Tp[:], G_sb[k2][:, kc * 128:(kc + 1) * 128],
                wq_sb[k2][:, 2 * C:3 * C],
                start=(k2 == 0), stop=(k2 == KC - 1),
            )
        nc.vector.tensor_copy(T_sb[kc][:, 0:C // 2], Tp[:, 0:C // 2])
        nc.scalar.activation(T_sb[kc][:, C // 2:C], Tp[:, C // 2:C], AF.Copy)

    # per-pair lg banks [128, 512] at f32r full rate (free-dim 512,
    # 1 cyc/col): the matmul computes pair-p's d rows against ALL
    # 512 v-columns; only head h's own 64-col block is ever read
    # (cols 64h), the rest is junk. Same cycles as the exact
    # 128-free variant would take at 4 cyc/col, but pairs complete
    # (stop=) individually so the softmax pipelines per pair.
    lgp = bank[0:HP]
    for p in range(HP):
        for kc in range(KC):
            nc.tensor.matmul(
                lgp[p][:],
                wq_sb[kc][:, C + p * 128:C + (p + 1) * 128],
                T_sb[kc][:],
                start=(kc == 0), stop=(kc == KC - 1),
            )
    if lvl < 3:
        return
    # softmax over each head's diag block, exp into block-diag bd[p]
    if first_rep:
        for p in range(HP):
            nc.gpsimd.memset(bd[p][:], 0.0)
    for p in range(HP):
        for par in range(2):
            psl = slice(64 * par, 64 * par + 64)
            csl = slice((2 * p + par) * 64, (2 * p + par) * 64 + 64)
            nc.vector.reduce_max(mx[psl, p:p + 1], lgp[p][psl, csl], axis=AX)
        nc.vector.tensor_scalar_mul(bias[:, p:p + 1], mx[:, p:p + 1], -SCALE)
        for par in range(2):
            psl = slice(64 * par, 64 * par + 64)
            csl = slice((2 * p + par) * 64, (2 * p + par) * 64 + 64)
            nc.scalar.activation(
                bd[p][psl, psl], lgp[p][psl, csl], AF.Exp,
                bias=bias[psl, p:p + 1], scale=SCALE,
            )
            nc.vector.reduce_sum(ssum[psl, p:p + 1], bd[p][psl, psl], axis=AX)
        nc.vector.reciprocal(recip[:, p:p + 1], ssum[:, p:p + 1])
        # fold 1/rowsum into the tiny exp matrix (rows d of head
        # 2p+par scaled by recip), not into the [128,512] w_proj
        nc.vector.tensor_scalar_mul(bd2[p][:], bd[p][:], recip[:, p:p + 1])

    if lvl < 4:
        return
    # Wq^T (data-independent, first pass only): emitted here — not at
    # kernel start — so the PE never stalls on the w_qkv DMA; it is
    # first consumed a few instructions later by the Wy matmuls.
    if first_rep:
        for kq in range(KC):
            tp = bank[4 + kq % 2]
            for kci in range(KC):
                nc.tensor.matmul(
                    _r(tp[:, kci * 128:(kci + 1) * 128]),
                    wq_sb[kci][:, kq * 128:(kq + 1) * 128],
                    id_sb[:], is_transpose=True,
                    start=(kci == 0), stop=(kci == KC - 1),
                )
            if kq % 2 == 0:
                nc.vector.tensor_copy(wqT_sb[kq][:], tp[:])
            else:
                nc.scalar.activation(wqT_sb[kq][:], tp[:], AF.Copy)

    # M^T[128p + 64par + e, c] = sum_d (E/s)[d,e] * wp[128p + 64par + d, c]
    # then immediately fold pair p into Wy[ci, co] = sum_q Wq[ci,q] M^T[q,co]
    wyps = bank[0:KC]
    for p in range(HP):
        mp = bank[4 + p % 2]
        nc.tensor.matmul(mp[:], bd2[p][:], wp_sb[p][:], start=True, stop=True)
        nc.vector.tensor_copy(mt_sb[p][:, 0:C // 2], mp[:, 0:C // 2])
        nc.scalar.activation(mt_sb[p][:, C // 2:C], mp[:, C // 2:C], AF.Copy)
        for ci in range(KC):
            nc.tensor.matmul(
                wyps[ci][:], wqT_sb[p][:, ci * 128:(ci + 1) * 128], mt_sb[p][:],
                start=(p == 0), stop=(p == HP - 1),
            )
    for ci in range(KC):
        nc.vector.tensor_copy(wy_sb[ci][:, 0:C // 2], wyps[ci][:, 0:C // 2])
        nc.scalar.activation(wy_sb[ci][:, C // 2:C], wyps[ci][:, C // 2:C], AF.Copy)

    if lvl < 5:
        return
    # ================= Phase C: y^T = Wy^T x + b ========================
    # banks 6,7 only: disjoint from the next rep's phase A banks.
    with tc.tile_pool(name="ys", bufs=4) as ypool:
        for ns in range(NS):
            nsl = slice(ns * SL, (ns + 1) * SL)
            for co in range(KC):
                yp = bank[6 + (ns * KC + co) % 2]
                for ci in range(KC):
                    nc.tensor.matmul(
                        yp[:], wy_sb[ci][:, co * 128:(co + 1) * 128],
                        x_sb[ci][ns][:],
                        start=(ci == 0), stop=(ci == KC - 1),
                    )
                ysb = ypool.tile([128, SL], F32, tag="y_sb")
                nc.scalar.activation(
                    ysb[:], yp[:], AF.Identity,
                    bias=bp_sb[co][:, 0:1], scale=1.0,
                )
                nc.sync.dma_start(yd[co * 128:(co + 1) * 128, nsl], ysb[:])


_NC_CACHE = None


def kernel(x, w_qkv, w_proj, b_proj, num_heads):
    x = np.ascontiguousarray(np.asarray(x, dtype=np.float32))
    w_qkv = np.ascontiguousarray(np.asarray(w_qkv, dtype=np.float32))
    w_proj = np.ascontiguousarray(np.asarray(w_proj, dtype=np.float32))
    b_proj = np.ascontiguousarray(np.asarray(b_proj, dtype=np.float32))
    assert int(num_heads) == NH
    assert x.shape == (B, C, H, W)

    xs = x.reshape(B, C, N)
    bp2 = b_proj.reshape(C, 1)
    in_maps = [
        {"x": xs[b], "w_qkv": w_qkv, "w_proj": w_proj, "b_proj": bp2}
        for b in range(B)
    ]
    global _NC_CACHE
    if _NC_CACHE is None:
        _NC_CACHE = build_nc()
    res = bass_utils.run_bass_kernel_spmd(_NC_CACHE, in_maps, list(range(B)))
    y = np.stack([res.results[b]["y"] for b in range(B)])
    return y.reshape(B, C, H, W).astype(np.float32)


if __name__ == "__main__":
    nc = build_nc()
    n_inst = sum(len(bb.instructions) for bb in nc.main_func.blocks)
    print(f"built OK, {n_inst} instructions")



# revision 13
# speedup vs baseline: 1.4351x; 1.4351x over previous
"""Channel-attention (transposed attention) Trainium2 Bass kernel.

Reference computation (per batch b of 8, one NeuronCore each):
    X    = x[b].reshape(C, N).T                    # [N, C], N = 64*64 = 4096
    qkv  = X @ w_qkv                               # [N, 3C]
    q, k, v : per-head [N, hd], nh=8, hd=64
    logits_h = k_h.T @ v_h                         # [hd, hd]
    attn_h   = softmax(scale * logits_h, axis=-1)  # scale = hd**-0.5 = 1/8
    out_h    = q_h @ attn_h.T                      # [N, hd]
    y[b] = (concat_h(out_h) @ w_proj + b_proj).T   # [C, N]

Sharding: data-parallel over batch, 1 batch item per core, no collectives.

Algebraic restructuring:
1. Gram trick. logits_h = Wk_h^T (X^T X) Wv_h. G = X^T X is one [C, C]
   matmul over the 4096 tokens (PE transposes of x feed it); then
   T = G @ Wv and lg = Wk^T T (pair-packed) are tiny. k/v are never
   materialized at [N, .] size.
2. Weight folding. y^T = Wy^T X^T with Wy = Wq @ M^T,
   M^T[64h+e, :] = sum_d E_h[d,e] * (w_proj[64h+d, :] / s_h[d]).
   Kills the q projection and the attention-apply at [N, .] size.

Scheduling (the point of this version):
- Phase A (transpose+Gram) is software-pipelined: transposes of tile
  t+1 are emitted before the Gram matmuls of tile t, so the PE never
  waits on the PSUM->SBUF xt copy.
- x is double-buffered in SBUF (2 x 8MB). The next rep's x loads start
  at the top of the current rep with no WAR race against this rep's
  phase C, killing the rep-boundary stall.
- All of phase B and phase C run on 2 rotating PSUM banks (6,7);
  banks 0-3 hold the Gram accumulators and 4,5 the transpose staging
  for the NEXT rep, whose transpose+Gram units are interleaved into
  phase B/C stall slots. In steady state the PE stream is a single
  gap-free sequence.

All large matmuls are float32r (fp32 bytes, FP22 multiply): 1 PE
cycle/column at free-dim >= 256. The softmax itself is exact fp32.
"""

import numpy as np

import concourse.bass as bass
import concourse.mybir as mybir
import concourse.tile as tile
from concourse import bass_utils

F32 = mybir.dt.float32
F32R = mybir.dt.float32r
AF = mybir.ActivationFunctionType
AX = mybir.AxisListType.X

# Problem shape (hardcoded per contest contract).
B = 8
C = 512
H = W = 64
N = H * W            # 4096 tokens per batch
NH = 8               # heads
HD = C // NH         # 64
SCALE = HD ** -0.5   # 1/8
KC = C // 128        # 4 chunks of 128 channels
NS = 8               # n-slices of 512 tokens
SL = N // NS         # 512
TT = SL // 128       # 4 token tiles of 128 per slice
NT = NS * TT         # 32 token tiles total
HP = NH // 2         # 4 head pairs


def _r(ap):
    return ap.bitcast(F32R)


def _split_multi_waits(nc, max_waits=1):
    """The walrus build in this container encodes at most one sync-wait
    command per instruction. Hoist excess waits onto same-engine NOPs
    immediately preceding the instruction."""
    n_split = 0
    for bb in nc.main_func.blocks:
        new_insts = []
        for ins in bb.instructions:
            si = ins.sync_info
            waits = list(si.on_wait) if si and si.on_wait else []
            if len(waits) > max_waits:
                extra, keep = waits[:-max_waits], waits[-max_waits:]
                while extra:
                    chunk, extra = extra[:max_waits], extra[max_waits:]
                    nop = mybir.InstNoOp(
                        name=nc.get_next_instruction_name(),
                        ins=[], outs=[],
                        engine=ins.engine,
                        sync_info=mybir.SyncInfo(on_wait=chunk, on_update=[]),
                    )
                    nc.register_instruction(nop)
                    new_insts.append(nop)
                    n_split += 1
                si.on_wait = keep
            new_insts.append(ins)
        bb.instructions[:] = new_insts
    return n_split


class _Sched:
    """Holds tiles + emission helpers for the interleaved schedule."""

    def __init__(self, nc, cpool, ypool, bank):
        self.nc = nc
        self.cpool = cpool
        self.ypool = ypool
        self.bank = bank
        self.rot_i = 0          # rotation over banks 6,7 for B/C stages

    def rot(self):
        b = self.bank[6 + self.rot_i % 2]
        self.rot_i += 1
        return b

    def copy2(self, dst, src, n=C):
        """2-way split PSUM->SBUF copy: DVE low half, ACT high half."""
        h = n // 2
        self.nc.vector.tensor_copy(dst[:, 0:h], src[:, 0:h])
        self.nc.scalar.activation(dst[:, h:n], src[:, h:n], AF.Copy)


def build_nc(reps=1, phases="full"):
    nc = bass.Bass("TRN2", debug=False, num_devices=B)

    x_t = nc.dram_tensor("x", [C, N], F32, kind="ExternalInput")
    wq_t = nc.dram_tensor("w_qkv", [C, 3 * C], F32, kind="ExternalInput")
    wp_t = nc.dram_tensor("w_proj", [C, C], F32, kind="ExternalInput")
    bp_t = nc.dram_tensor("b_proj", [C, 1], F32, kind="ExternalInput")
    y_t = nc.dram_tensor("y", [C, N], F32, kind="ExternalOutput")
    id_t = nc.inline_tensor(np.eye(128, dtype=np.float32), name="id128")

    xd, wqd, wpd, bpd, yd = x_t.ap(), wq_t.ap(), wp_t.ap(), bp_t.ap(), y_t.ap()

    with tile.TileContext(nc) as tc:
        with (
            tc.tile_pool(name="const", bufs=1) as cpool,
            tc.tile_pool(name="ys", bufs=4) as ypool,
            tc.tile_pool(name="ps", bufs=1, space="PSUM") as pspool,
        ):
            bank = [pspool.tile([128, C], F32, name=f"bank{i}", tag=f"bank{i}")
                    for i in range(8)]
            S = _Sched(nc, cpool, ypool, bank)

            # ---------------- persistent tiles -------------------------
            # double-buffered resident x: 2 x [4 chunks][8 slices] of
            # [128, 512] f32r  (2 x 8MB)
            x_sb = [
                [[cpool.tile([128, SL], F32R, name=f"x{bf}_{k}_{ns}",
                             tag=f"x{bf}_{k}_{ns}")
                  for ns in range(NS)] for k in range(KC)]
                for bf in range(2)
            ]
            id_sb = cpool.tile([128, 128], F32R, tag="id")
            # k/v sections of w_qkv: k in cols [0:512], v in [512:1024]
            wkv_sb = [cpool.tile([128, 2 * C], F32R, name=f"wkv{k}",
                                 tag=f"wkv{k}") for k in range(KC)]
            wp_sb = [cpool.tile([128, C], F32R, name=f"wp{k}", tag=f"wp{k}")
                     for k in range(KC)]
            bp_sb = [cpool.tile([128, 1], F32, name=f"bp{k}", tag=f"bp{k}")
                     for k in range(KC)]
            wqT_sb = [cpool.tile([128, C], F32R, name=f"wqT{k}", tag=f"wqT{k}")
                      for k in range(KC)]
            G_sb = [cpool.tile([128, C], F32R, name=f"G{k}", tag=f"G{k}")
                    for k in range(KC)]
            T_sb = [cpool.tile([128, C], F32R, name=f"T{k}", tag=f"T{k}")
                    for k in range(KC)]
            # M^T reuses T's storage: T is dead after the lg stage, and
            # Tile's WAR tracking orders the mt copies behind lg's reads.
            mt_sb = T_sb
            wy_sb = [cpool.tile([128, C], F32R, name=f"wy{k}", tag=f"wy{k}")
                     for k in range(KC)]
            xt_sb = [cpool.tile([128, C], F32R, name=f"xt{i}", tag=f"xt{i}")
                     for i in range(2)]
            bd = [cpool.tile([128, 128], F32, name=f"bd{p}", tag=f"bd{p}")
                  for p in range(HP)]
            bd2 = [cpool.tile([128, 128], F32R, name=f"bd2{p}", tag=f"bd2{p}")
                   for p in range(HP)]
            mx = cpool.tile([128, HP], F32, tag="mx")
            sbias = cpool.tile([128, HP], F32, tag="sbias")
            ssum = cpool.tile([128, HP], F32, tag="ssum")
            recip = cpool.tile([128, HP], F32, tag="recip")

            # ---------------- prologue DMAs ----------------------------
            nc.scalar.dma_start(id_sb[:], _r(id_t.ap()[:, :]))
            _emit_x_loads(nc, x_sb[0], xd)
            # weights: behind x(0) on the rings; first needed ~25us in
            for k in range(KC):
                r = slice(k * 128, (k + 1) * 128)
                eng = nc.sync if k % 2 == 0 else nc.scalar
                eng.dma_start(wkv_sb[k][:, 0:C], _r(wqd[r, C:2 * C]))
                eng.dma_start(wkv_sb[k][:, C:2 * C], _r(wqd[r, 2 * C:3 * C]))
            # q section into recycled ypool tiles (freed for phase C use)
            wq_q = []
            for k in range(KC):
                r = slice(k * 128, (k + 1) * 128)
                qt = ypool.tile([128, C], F32, tag="y_sb")
                eng = nc.sync if k % 2 == 0 else nc.scalar
                eng.dma_start(_r(qt[:]), _r(wqd[r, 0:C]))
                wq_q.append(qt)
            for k in range(KC):
                r = slice(k * 128, (k + 1) * 128)
                eng = nc.sync if k % 2 == 0 else nc.scalar
                eng.dma_start(wp_sb[k][:], _r(wpd[r, :]))
                eng.dma_start(bp_sb[k][:], bpd[r, :])
            # exp writes only the diagonal blocks of bd; the off-diagonal
            # zeros propagate into bd2 via the full-tile rowsum scaling
            for p in range(HP):
                nc.gpsimd.memset(bd[p][:], 0.0)

            # ---------------- prologue compute -------------------------
            # rep 0 phase A, standalone but software-pipelined
            for u in _a_units(S, x_sb[0], id_sb, xt_sb, G_sb):
                u()
            # Wq^T build (data-independent, once): staging banks 4,5
            for kq in range(KC):
                tp = bank[4 + kq % 2]
                for kci in range(KC):
                    nc.tensor.matmul(
                        _r(tp[:, kci * 128:(kci + 1) * 128]),
                        _r(wq_q[kci][:, kq * 128:(kq + 1) * 128]),
                        id_sb[:], is_transpose=True,
                        start=(kci == 0), stop=(kci == KC - 1),
                    )
                S.copy2(wqT_sb[kq], tp)

            # ---------------- steady-state rep loop --------------------
            for r in range(reps):
                nxt = (r + 1) % 2
                slots = []
                if r + 1 < reps:
                    _emit_x_loads(nc, x_sb[nxt], xd)
                    slots = _a_units(S, x_sb[nxt], id_sb, xt_sb, G_sb)
                slots = list(slots)
                si = 0

                def pull(n=1):
                    nonlocal si
                    for _ in range(n):
                        if si < len(slots):
                            slots[si]()
                            si += 1

                _emit_bc(nc, S, pull, wkv_sb, wp_sb, bp_sb, wqT_sb, G_sb,
                         T_sb, mt_sb, wy_sb, bd, bd2, mx, sbias, ssum, recip,
                         x_sb[r % 2], yd)
                pull(len(slots))  # drain any leftovers

    _split_multi_waits(nc)
    return nc


def _emit_x_loads(nc, xbuf, xd):
    """Load one full x image into an SBUF buffer, alternating rings."""
    for ns in range(NS):
        nsl = slice(ns * SL, (ns + 1) * SL)
        for k in range(KC):
            eng = nc.sync if (ns * KC + k) % 2 == 0 else nc.scalar
            eng.dma_start(xbuf[k][ns][:], _r(xd[k * 128:(k + 1) * 128, nsl]))


def _a_units(S, xbuf, id_sb, xt_sb, G_sb):
    """33 closures: unit i = [transposes of tile i][gram of tile i-1].
    Unit 32 is gram-only + the G PSUM->SBUF copies."""
    nc = S.nc
    units = []

    def mk(i):
        def unit():
            if i < NT:
                ns, t = divmod(i, TT)
                tsl = slice(t * 128, (t + 1) * 128)
                tp = S.bank[4 + i % 2]
                for k in range(KC):
                    nc.tensor.matmul(
                        _r(tp[:, k * 128:(k + 1) * 128]),
                        xbuf[k][ns][:, tsl], id_sb[:], is_transpose=True,
                        start=(k == 0), stop=(k == KC - 1),
                    )
                S.copy2(xt_sb[i % 2], tp)
            if i >= 1:
                j = i - 1
                xt = xt_sb[j % 2]
                for k in range(KC):
                    nc.tensor.matmul(
                        S.bank[k][:], xt[:, k * 128:(k + 1) * 128], xt[:],
                        start=(j == 0), stop=(j == NT - 1),
                    )
            if i == NT:
                for k in range(KC):
                    S.copy2(G_sb[k], S.bank[k])
        return unit

    for i in range(NT + 1):
        units.append(mk(i))
    return units


def _emit_bc(nc, S, pull, wkv_sb, wp_sb, bp_sb, wqT_sb, G_sb, T_sb, mt_sb,
             wy_sb, bd, bd2, mx, sbias, ssum, recip, xbuf, yd):
    # ---------------- phase B ------------------------------------------
    # T = G @ Wv   (v section = wkv cols [C:2C])
    for kc in range(KC):
        Tp = S.rot()
        for k2 in range(KC):
            nc.tensor.matmul(
                Tp[:], G_sb[k2][:, kc * 128:(kc + 1) * 128],
                wkv_sb[k2][:, C:2 * C],
                start=(k2 == 0), stop=(k2 == KC - 1),
            )
        S.copy2(T_sb[kc], Tp)

    # lg pairs: [128, 512] at full f32r rate; only each head's own
    # 64-col diagonal block is meaningful (rest junk, never read)
    lgp = []
    for p in range(HP):
        Lp = S.rot()
        lgp.append(Lp)
        for kc in range(KC):
            nc.tensor.matmul(
                Lp[:], wkv_sb[kc][:, p * 128:(p + 1) * 128], T_sb[kc][:],
                start=(kc == 0), stop=(kc == KC - 1),
            )
        if p == 1 or p == 3:
            pull()
        # softmax for this pair (reads PSUM directly)
        for par in range(2):
            psl = slice(64 * par, 64 * par + 64)
            csl = slice((2 * p + par) * 64, (2 * p + par) * 64 + 64)
            nc.vector.reduce_max(mx[psl, p:p + 1], Lp[psl, csl], axis=AX)
        nc.vector.tensor_scalar_mul(sbias[:, p:p + 1], mx[:, p:p + 1], -SCALE)
        for par in range(2):
            psl = slice(64 * par, 64 * par + 64)
            csl = slice((2 * p + par) * 64, (2 * p + par) * 64 + 64)
            nc.scalar.activation(
                bd[p][psl, psl], Lp[psl, csl], AF.Exp,
                bias=sbias[psl, p:p + 1], scale=SCALE,
            )
            nc.vector.reduce_sum(ssum[psl, p:p + 1], bd[p][psl, psl], axis=AX)
        nc.vector.reciprocal(recip[:, p:p + 1], ssum[:, p:p + 1])
        # fold 1/rowsum into the tiny exp matrix
        nc.vector.tensor_scalar_mul(bd2[p][:], bd[p][:], recip[:, p:p + 1])

    # M^T per pair, then Wy ci-sequential over 2 rotating banks
    for p in range(HP):
        mp = S.rot()
        nc.tensor.matmul(mp[:], bd2[p][:], wp_sb[p][:],
                         start=True, stop=True)
        S.copy2(mt_sb[p], mp)
        if p == 1 or p == 3:
            pull()
    for ci in range(KC):
        wyb = S.rot()
        for p in range(HP):
            nc.tensor.matmul(
                wyb[:], wqT_sb[p][:, ci * 128:(ci + 1) * 128], mt_sb[p][:],
                start=(p == 0), stop=(p == HP - 1),
            )
        S.copy2(wy_sb[ci], wyb)
        if ci == 1 or ci == 3:
            pull()

    # ---------------- phase C: y^T = Wy^T x + b ------------------------
    for ns in range(NS):
        nsl = slice(ns * SL, (ns + 1) * SL)
        for co in range(KC):
            yp = S.rot()
            for ci in range(KC):
                nc.tensor.matmul(
                    yp[:], wy_sb[ci][:, co * 128:(co + 1) * 128],
                    xbuf[ci][ns][:],
                    start=(ci == 0), stop=(ci == KC - 1),
                )
            ysb = S.ypool.tile([128, SL], F32, tag="y_sb")
            nc.scalar.activation(
                ysb[:], yp[:], AF.Identity,
                bias=bp_sb[co][:, 0:1], scale=1.0,
            )
            eng = nc.sync if (ns * KC + co) % 2 == 0 else nc.scalar
            eng.dma_start(yd[co * 128:(co + 1) * 128, nsl], ysb[:])
            pull()


_NC_CACHE = None


def kernel(x, w_qkv, w_proj, b_proj, num_heads):
    x = np.ascontiguousarray(np.asarray(x, dtype=np.float32))
    w_qkv = np.ascontiguousarray(np.asarray(w_qkv, dtype=np.float32))
    w_proj = np.ascontiguousarray(np.asarray(w_proj, dtype=np.float32))
    b_proj = np.ascontiguousarray(np.asarray(b_proj, dtype=np.float32))
    assert int(num_heads) == NH
    assert x.shape == (B, C, H, W)

    xs = x.reshape(B, C, N)
    bp2 = b_proj.reshape(C, 1)
    in_maps = [
        {"x": xs[b], "w_qkv": w_qkv, "w_proj": w_proj, "b_proj": bp2}
        for b in range(B)
    ]
    global _NC_CACHE
    if _NC_CACHE is None:
        _NC_CACHE = build_nc()
    res = bass_utils.run_bass_kernel_spmd(_NC_CACHE, in_maps, list(range(B)))
    y = np.stack([res.results[b]["y"] for b in range(B)])
    return y.reshape(B, C, H, W).astype(np.float32)


if __name__ == "__main__":
    nc = build_nc(reps=2)
    n_inst = sum(len(bb.instructions) for bb in nc.main_func.blocks)
    print(f"built OK, {n_inst} instructions")


# revision 15
# speedup vs baseline: 1.4356x; 1.0003x over previous
"""Channel-attention (transposed attention) Trainium2 Bass kernel.

Reference computation (per batch b of 8, one NeuronCore each):
    X    = x[b].reshape(C, N).T                    # [N, C], N = 64*64 = 4096
    qkv  = X @ w_qkv                               # [N, 3C]
    q, k, v : per-head [N, hd], nh=8, hd=64
    logits_h = k_h.T @ v_h                         # [hd, hd]
    attn_h   = softmax(scale * logits_h, axis=-1)  # scale = hd**-0.5 = 1/8
    out_h    = q_h @ attn_h.T                      # [N, hd]
    y[b] = (concat_h(out_h) @ w_proj + b_proj).T   # [C, N]

Sharding: data-parallel over batch, 1 batch item per core, no collectives.

Algebraic restructuring:
1. Gram trick. logits_h = Wk_h^T (X^T X) Wv_h. G = X^T X is one [C, C]
   matmul over the 4096 tokens (PE transposes of x feed it); then
   T = G @ Wv and lg = Wk^T T (pair-packed) are tiny. k/v are never
   materialized at [N, .] size.
2. Weight folding. y^T = Wy^T X^T with Wy = Wq @ M^T,
   M^T[64h+e, :] = sum_d E_h[d,e] * (w_proj[64h+d, :] / s_h[d]).
   Kills the q projection and the attention-apply at [N, .] size.

Scheduling (the point of this version):
- Phase A (transpose+Gram) is software-pipelined: transposes of tile
  t+1 are emitted before the Gram matmuls of tile t, so the PE never
  waits on the PSUM->SBUF xt copy.
- x is double-buffered in SBUF (2 x 8MB). The next rep's x loads start
  at the top of the current rep with no WAR race against this rep's
  phase C, killing the rep-boundary stall.
- All of phase B and phase C run on 2 rotating PSUM banks (6,7);
  banks 0-3 hold the Gram accumulators and 4,5 the transpose staging
  for the NEXT rep, whose transpose+Gram units are interleaved into
  phase B/C stall slots. In steady state the PE stream is a single
  gap-free sequence.

All large matmuls are float32r (fp32 bytes, FP22 multiply): 1 PE
cycle/column at free-dim >= 256. The softmax itself is exact fp32.
"""

import numpy as np

import concourse.bass as bass
import concourse.mybir as mybir
import concourse.tile as tile
from concourse import bass_utils

F32 = mybir.dt.float32
F32R = mybir.dt.float32r
AF = mybir.ActivationFunctionType
AX = mybir.AxisListType.X

# Problem shape (hardcoded per contest contract).
B = 8
C = 512
H = W = 64
N = H * W            # 4096 tokens per batch
NH = 8               # heads
HD = C // NH         # 64
SCALE = HD ** -0.5   # 1/8
KC = C // 128        # 4 chunks of 128 channels
NS = 8               # n-slices of 512 tokens
SL = N // NS         # 512
TT = SL // 128       # 4 token tiles of 128 per slice
NT = NS * TT         # 32 token tiles total
HP = NH // 2         # 4 head pairs


def _r(ap):
    return ap.bitcast(F32R)


def _split_multi_waits(nc, max_waits=1):
    """The walrus build in this container encodes at most one sync-wait
    command per instruction. Hoist excess waits onto same-engine NOPs
    immediately preceding the instruction."""
    n_split = 0
    for bb in nc.main_func.blocks:
        new_insts = []
        for ins in bb.instructions:
            si = ins.sync_info
            waits = list(si.on_wait) if si and si.on_wait else []
            if len(waits) > max_waits:
                extra, keep = waits[:-max_waits], waits[-max_waits:]
                while extra:
                    chunk, extra = extra[:max_waits], extra[max_waits:]
                    nop = mybir.InstNoOp(
                        name=nc.get_next_instruction_name(),
                        ins=[], outs=[],
                        engine=ins.engine,
                        sync_info=mybir.SyncInfo(on_wait=chunk, on_update=[]),
                    )
                    nc.register_instruction(nop)
                    new_insts.append(nop)
                    n_split += 1
                si.on_wait = keep
            new_insts.append(ins)
        bb.instructions[:] = new_insts
    return n_split


class _Sched:
    """Holds tiles + emission helpers for the interleaved schedule."""

    def __init__(self, nc, cpool, ypool, bank):
        self.nc = nc
        self.cpool = cpool
        self.ypool = ypool
        self.bank = bank
        self.rot_i = 0          # rotation over banks 6,7 for B/C stages

    def rot(self):
        b = self.bank[6 + self.rot_i % 2]
        self.rot_i += 1
        return b

    def copy2(self, dst, src, n=C):
        """2-way split PSUM->SBUF copy: DVE low half, ACT high half."""
        h = n // 2
        self.nc.vector.tensor_copy(dst[:, 0:h], src[:, 0:h])
        self.nc.scalar.activation(dst[:, h:n], src[:, h:n], AF.Copy)


def build_nc(reps=1, phases="full"):
    nc = bass.Bass("TRN2", debug=False, num_devices=B)

    x_t = nc.dram_tensor("x", [C, N], F32, kind="ExternalInput")
    wq_t = nc.dram_tensor("w_qkv", [C, 3 * C], F32, kind="ExternalInput")
    wp_t = nc.dram_tensor("w_proj", [C, C], F32, kind="ExternalInput")
    bp_t = nc.dram_tensor("b_proj", [C, 1], F32, kind="ExternalInput")
    y_t = nc.dram_tensor("y", [C, N], F32, kind="ExternalOutput")
    id_t = nc.inline_tensor(np.eye(128, dtype=np.float32), name="id128")

    xd, wqd, wpd, bpd, yd = x_t.ap(), wq_t.ap(), wp_t.ap(), bp_t.ap(), y_t.ap()

    with tile.TileContext(nc) as tc:
        with (
            tc.tile_pool(name="const", bufs=1) as cpool,
            tc.tile_pool(name="ys", bufs=6) as ypool,
            tc.tile_pool(name="ps", bufs=1, space="PSUM") as pspool,
        ):
            bank = [pspool.tile([128, C], F32, name=f"bank{i}", tag=f"bank{i}")
                    for i in range(8)]
            S = _Sched(nc, cpool, ypool, bank)

            # ---------------- persistent tiles -------------------------
            # double-buffered resident x: 2 x [4 chunks][8 slices] of
            # [128, 512] f32r  (2 x 8MB)
            x_sb = [
                [[cpool.tile([128, SL], F32R, name=f"x{bf}_{k}_{ns}",
                             tag=f"x{bf}_{k}_{ns}")
                  for ns in range(NS)] for k in range(KC)]
                for bf in range(2)
            ]
            id_sb = cpool.tile([128, 128], F32R, tag="id")
            # k/v sections of w_qkv: k in cols [0:512], v in [512:1024]
            wkv_sb = [cpool.tile([128, 2 * C], F32R, name=f"wkv{k}",
                                 tag=f"wkv{k}") for k in range(KC)]
            wp_sb = [cpool.tile([128, C], F32R, name=f"wp{k}", tag=f"wp{k}")
                     for k in range(KC)]
            bp_sb = [cpool.tile([128, 1], F32, name=f"bp{k}", tag=f"bp{k}")
                     for k in range(KC)]
            wqT_sb = [cpool.tile([128, C], F32R, name=f"wqT{k}", tag=f"wqT{k}")
                      for k in range(KC)]
            G_sb = [cpool.tile([128, C], F32R, name=f"G{k}", tag=f"G{k}")
                    for k in range(KC)]
            T_sb = [cpool.tile([128, C], F32R, name=f"T{k}", tag=f"T{k}")
                    for k in range(KC)]
            # M^T reuses T's storage: T is dead after the lg stage, and
            # Tile's WAR tracking orders the mt copies behind lg's reads.
            mt_sb = T_sb
            wy_sb = [cpool.tile([128, C], F32R, name=f"wy{k}", tag=f"wy{k}")
                     for k in range(KC)]
            xt_sb = [cpool.tile([128, C], F32R, name=f"xt{i}", tag=f"xt{i}")
                     for i in range(2)]
            bd = [cpool.tile([128, 128], F32, name=f"bd{p}", tag=f"bd{p}")
                  for p in range(HP)]
            bd2 = [cpool.tile([128, 128], F32R, name=f"bd2{p}", tag=f"bd2{p}")
                   for p in range(HP)]
            mx = cpool.tile([128, HP], F32, tag="mx")
            sbias = cpool.tile([128, HP], F32, tag="sbias")
            ssum = cpool.tile([128, HP], F32, tag="ssum")
            recip = cpool.tile([128, HP], F32, tag="recip")

            # ---------------- prologue DMAs ----------------------------
            nc.scalar.dma_start(id_sb[:], _r(id_t.ap()[:, :]))
            _emit_x_loads(nc, x_sb[0], xd)
            # weights: behind x(0) on the rings; first needed ~25us in
            for k in range(KC):
                r = slice(k * 128, (k + 1) * 128)
                eng = nc.sync if k % 2 == 0 else nc.scalar
                eng.dma_start(wkv_sb[k][:, 0:C], _r(wqd[r, C:2 * C]))
                eng.dma_start(wkv_sb[k][:, C:2 * C], _r(wqd[r, 2 * C:3 * C]))
            # q section into recycled ypool tiles (freed for phase C use)
            wq_q = []
            for k in range(KC):
                r = slice(k * 128, (k + 1) * 128)
                qt = ypool.tile([128, C], F32, tag="y_sb")
                eng = nc.sync if k % 2 == 0 else nc.scalar
                eng.dma_start(_r(qt[:]), _r(wqd[r, 0:C]))
                wq_q.append(qt)
            for k in range(KC):
                r = slice(k * 128, (k + 1) * 128)
                eng = nc.sync if k % 2 == 0 else nc.scalar
                eng.dma_start(wp_sb[k][:], _r(wpd[r, :]))
                eng.dma_start(bp_sb[k][:], bpd[r, :])
            # exp writes only the diagonal blocks of bd; the off-diagonal
            # zeros propagate into bd2 via the full-tile rowsum scaling
            for p in range(HP):
                nc.gpsimd.memset(bd[p][:], 0.0)

            # ---------------- prologue compute -------------------------
            # rep 0 phase A, standalone but software-pipelined
            for u in _a_units(S, x_sb[0], id_sb, xt_sb, G_sb):
                u()
            # Wq^T build (data-independent, once): staging banks 4,5
            for kq in range(KC):
                tp = bank[4 + kq % 2]
                for kci in range(KC):
                    nc.tensor.matmul(
                        _r(tp[:, kci * 128:(kci + 1) * 128]),
                        _r(wq_q[kci][:, kq * 128:(kq + 1) * 128]),
                        id_sb[:], is_transpose=True,
                        start=(kci == 0), stop=(kci == KC - 1),
                    )
                S.copy2(wqT_sb[kq], tp)

            # ---------------- steady-state rep loop --------------------
            for r in range(reps):
                nxt = (r + 1) % 2
                slots = []
                if r + 1 < reps:
                    _emit_x_loads(nc, x_sb[nxt], xd)
                    slots = _a_units(S, x_sb[nxt], id_sb, xt_sb, G_sb)
                slots = list(slots)
                si = 0

                def pull(n=1):
                    nonlocal si
                    for _ in range(n):
                        if si < len(slots):
                            slots[si]()
                            si += 1

                _emit_bc(nc, S, pull, wkv_sb, wp_sb, bp_sb, wqT_sb, G_sb,
                         T_sb, mt_sb, wy_sb, bd, bd2, mx, sbias, ssum, recip,
                         x_sb[r % 2], yd)
                pull(len(slots))  # drain any leftovers

    _split_multi_waits(nc)
    return nc


def _emit_x_loads(nc, xbuf, xd):
    """Load one full x image into an SBUF buffer, alternating rings."""
    for ns in range(NS):
        nsl = slice(ns * SL, (ns + 1) * SL)
        for k in range(KC):
            eng = nc.sync if (ns * KC + k) % 2 == 0 else nc.scalar
            eng.dma_start(xbuf[k][ns][:], _r(xd[k * 128:(k + 1) * 128, nsl]))


def _a_units(S, xbuf, id_sb, xt_sb, G_sb):
    """33 closures: unit i = [transposes of tile i][gram of tile i-1].
    Unit 32 is gram-only + the G PSUM->SBUF copies."""
    nc = S.nc
    units = []

    def mk(i):
        def unit():
            if i < NT:
                ns, t = divmod(i, TT)
                tsl = slice(t * 128, (t + 1) * 128)
                tp = S.bank[4 + i % 2]
                for k in range(KC):
                    nc.tensor.matmul(
                        _r(tp[:, k * 128:(k + 1) * 128]),
                        xbuf[k][ns][:, tsl], id_sb[:], is_transpose=True,
                        start=(k == 0), stop=(k == KC - 1),
                    )
                S.copy2(xt_sb[i % 2], tp)
            if i >= 1:
                j = i - 1
                xt = xt_sb[j % 2]
                for k in range(KC):
                    nc.tensor.matmul(
                        S.bank[k][:], xt[:, k * 128:(k + 1) * 128], xt[:],
                        start=(j == 0), stop=(j == NT - 1),
                    )
            if i == NT:
                for k in range(KC):
                    S.copy2(G_sb[k], S.bank[k])
        return unit

    for i in range(NT + 1):
        units.append(mk(i))
    return units


def _emit_bc(nc, S, pull, wkv_sb, wp_sb, bp_sb, wqT_sb, G_sb, T_sb, mt_sb,
             wy_sb, bd, bd2, mx, sbias, ssum, recip, xbuf, yd):
    # ---------------- phase B ------------------------------------------
    # T = G @ Wv   (v section = wkv cols [C:2C])
    for kc in range(KC):
        Tp = S.rot()
        for k2 in range(KC):
            nc.tensor.matmul(
                Tp[:], G_sb[k2][:, kc * 128:(kc + 1) * 128],
                wkv_sb[k2][:, C:2 * C],
                start=(k2 == 0), stop=(k2 == KC - 1),
            )
        S.copy2(T_sb[kc], Tp)

    # lg pairs: [128, 512] at full f32r rate; only each head's own
    # 64-col diagonal block is meaningful (rest junk, never read)
    lgp = []
    for p in range(HP):
        Lp = S.rot()
        lgp.append(Lp)
        for kc in range(KC):
            nc.tensor.matmul(
                Lp[:], wkv_sb[kc][:, p * 128:(p + 1) * 128], T_sb[kc][:],
                start=(kc == 0), stop=(kc == KC - 1),
            )
        if p == 1 or p == 3:
            pull()
        # softmax for this pair (reads PSUM directly)
        for par in range(2):
            psl = slice(64 * par, 64 * par + 64)
            csl = slice((2 * p + par) * 64, (2 * p + par) * 64 + 64)
            nc.vector.reduce_max(mx[psl, p:p + 1], Lp[psl, csl], axis=AX)
        nc.vector.tensor_scalar_mul(sbias[:, p:p + 1], mx[:, p:p + 1], -SCALE)
        for par in range(2):
            psl = slice(64 * par, 64 * par + 64)
            csl = slice((2 * p + par) * 64, (2 * p + par) * 64 + 64)
            nc.scalar.activation(
                bd[p][psl, psl], Lp[psl, csl], AF.Exp,
                bias=sbias[psl, p:p + 1], scale=SCALE,
            )
            nc.vector.reduce_sum(ssum[psl, p:p + 1], bd[p][psl, psl], axis=AX)
        nc.vector.reciprocal(recip[:, p:p + 1], ssum[:, p:p + 1])
        # fold 1/rowsum into the tiny exp matrix
        nc.vector.tensor_scalar_mul(bd2[p][:], bd[p][:], recip[:, p:p + 1])

    # M^T per pair, then Wy ci-sequential over 2 rotating banks
    for p in range(HP):
        mp = S.rot()
        nc.tensor.matmul(mp[:], bd2[p][:], wp_sb[p][:],
                         start=True, stop=True)
        S.copy2(mt_sb[p], mp)
        if p == 1 or p == 3:
            pull()
    for ci in range(KC):
        wyb = S.rot()
        for p in range(HP):
            nc.tensor.matmul(
                wyb[:], wqT_sb[p][:, ci * 128:(ci + 1) * 128], mt_sb[p][:],
                start=(p == 0), stop=(p == HP - 1),
            )
        S.copy2(wy_sb[ci], wyb)
        if ci == 1 or ci == 3:
            pull()

    # ---------------- phase C: y^T = Wy^T x + b ------------------------
    for ns in range(NS):
        nsl = slice(ns * SL, (ns + 1) * SL)
        for co in range(KC):
            yp = S.rot()
            for ci in range(KC):
                nc.tensor.matmul(
                    yp[:], wy_sb[ci][:, co * 128:(co + 1) * 128],
                    xbuf[ci][ns][:],
                    start=(ci == 0), stop=(ci == KC - 1),
                )
            ysb = S.ypool.tile([128, SL], F32, tag="y_sb")
            g = ns * KC + co
            if g % 2 == 0:
                nc.scalar.activation(
                    ysb[:], yp[:], AF.Identity,
                    bias=bp_sb[co][:, 0:1], scale=1.0,
                )
            else:
                nc.vector.tensor_scalar_add(ysb[:], yp[:], bp_sb[co][:, 0:1])
            eng = nc.sync if g % 2 == 0 else nc.scalar
            eng.dma_start(yd[co * 128:(co + 1) * 128, nsl], ysb[:])
            pull()


_NC_CACHE = None


def kernel(x, w_qkv, w_proj, b_proj, num_heads):
    x = np.ascontiguousarray(np.asarray(x, dtype=np.float32))
    w_qkv = np.ascontiguousarray(np.asarray(w_qkv, dtype=np.float32))
    w_proj = np.ascontiguousarray(np.asarray(w_proj, dtype=np.float32))
    b_proj = np.ascontiguousarray(np.asarray(b_proj, dtype=np.float32))
    assert int(num_heads) == NH
    assert x.shape == (B, C, H, W)

    xs = x.reshape(B, C, N)
    bp2 = b_proj.reshape(C, 1)
    in_maps = [
        {"x": xs[b], "w_qkv": w_qkv, "w_proj": w_proj, "b_proj": bp2}
        for b in range(B)
    ]
    global _NC_CACHE
    if _NC_CACHE is None:
        _NC_CACHE = build_nc()
    res = bass_utils.run_bass_kernel_spmd(_NC_CACHE, in_maps, list(range(B)))
    y = np.stack([res.results[b]["y"] for b in range(B)])
    return y.reshape(B, C, H, W).astype(np.float32)


if __name__ == "__main__":
    nc = build_nc(reps=2)
    n_inst = sum(len(bb.instructions) for bb in nc.main_func.blocks)
    print(f"built OK, {n_inst} instructions")


# revision 20
# speedup vs baseline: 1.5324x; 1.0674x over previous
"""Channel-attention (transposed attention) Trainium2 Bass kernel.

Reference computation (per batch b of 8, one NeuronCore each):
    X    = x[b].reshape(C, N).T                    # [N, C], N = 64*64 = 4096
    qkv  = X @ w_qkv                               # [N, 3C]
    q, k, v : per-head [N, hd], nh=8, hd=64
    logits_h = k_h.T @ v_h                         # [hd, hd]
    attn_h   = softmax(scale * logits_h, axis=-1)  # scale = hd**-0.5 = 1/8
    out_h    = q_h @ attn_h.T                      # [N, hd]
    y[b] = (concat_h(out_h) @ w_proj + b_proj).T   # [C, N]

Sharding: data-parallel over batch, 1 batch item per core, no collectives.

Algebraic restructuring:
1. Gram trick. logits_h = Wk_h^T (X^T X) Wv_h. G = X^T X is one [C, C]
   matmul over the 4096 tokens (PE transposes of x feed it); then
   T = G @ Wv and lg = Wk^T T (pair-packed) are tiny. k/v are never
   materialized at [N, .] size.
2. Weight folding. y^T = Wy^T X^T with Wy = Wq @ M^T,
   M^T[64h+e, :] = sum_d E_h[d,e] * (w_proj[64h+d, :] / s_h[d]).
   Kills the q projection and the attention-apply at [N, .] size.

Scheduling (the point of this version):
- Phase A (transpose+Gram) is software-pipelined: transposes of tile
  t+1 are emitted before the Gram matmuls of tile t, so the PE never
  waits on the PSUM->SBUF xt copy.
- x is double-buffered in SBUF (2 x 8MB). The next rep's x loads start
  at the top of the current rep with no WAR race against this rep's
  phase C, killing the rep-boundary stall.
- All of phase B and phase C run on 2 rotating PSUM banks (6,7);
  banks 0-3 hold the Gram accumulators and 4,5 the transpose staging
  for the NEXT rep, whose transpose+Gram units are interleaved into
  phase B/C stall slots. In steady state the PE stream is a single
  gap-free sequence.

All large matmuls are float32r (fp32 bytes, FP22 multiply): 1 PE
cycle/column at free-dim >= 256. The softmax itself is exact fp32.
"""

import numpy as np

import concourse.bass as bass
import concourse.mybir as mybir
import concourse.tile as tile
from concourse import bass_utils

F32 = mybir.dt.float32
F32R = mybir.dt.float32r
AF = mybir.ActivationFunctionType
AX = mybir.AxisListType.X

# Problem shape (hardcoded per contest contract).
B = 8
C = 512
H = W = 64
N = H * W            # 4096 tokens per batch
NH = 8               # heads
HD = C // NH         # 64
SCALE = HD ** -0.5   # 1/8
KC = C // 128        # 4 chunks of 128 channels
NS = 8               # n-slices of 512 tokens
SL = N // NS         # 512
TT = SL // 128       # 4 token tiles of 128 per slice
NT = NS * TT         # 32 token tiles total
HP = NH // 2         # 4 head pairs


def _r(ap):
    return ap.bitcast(F32R)


def _split_multi_waits(nc, max_waits=1):
    """The walrus build in this container encodes at most one sync-wait
    command per instruction. Hoist excess waits onto same-engine NOPs
    immediately preceding the instruction."""
    n_split = 0
    for bb in nc.main_func.blocks:
        new_insts = []
        for ins in bb.instructions:
            si = ins.sync_info
            waits = list(si.on_wait) if si and si.on_wait else []
            if len(waits) > max_waits:
                extra, keep = waits[:-max_waits], waits[-max_waits:]
                while extra:
                    chunk, extra = extra[:max_waits], extra[max_waits:]
                    nop = mybir.InstNoOp(
                        name=nc.get_next_instruction_name(),
                        ins=[], outs=[],
                        engine=ins.engine,
                        sync_info=mybir.SyncInfo(on_wait=chunk, on_update=[]),
                    )
                    nc.register_instruction(nop)
                    new_insts.append(nop)
                    n_split += 1
                si.on_wait = keep
            new_insts.append(ins)
        bb.instructions[:] = new_insts
    return n_split


class _Sched:
    """Holds tiles + emission helpers for the interleaved schedule."""

    def __init__(self, nc, cpool, ypool, bank):
        self.nc = nc
        self.cpool = cpool
        self.ypool = ypool
        self.bank = bank
        self.rot_i = 0          # rotation over banks 6,7 for B/C stages

    def rot(self):
        b = self.bank[6 + self.rot_i % 2]
        self.rot_i += 1
        return b

    def copy2(self, dst, src, n=C):
        """2-way split PSUM->SBUF copy: DVE low half, ACT high half."""
        h = n // 2
        self.nc.vector.tensor_copy(dst[:, 0:h], src[:, 0:h])
        self.nc.scalar.activation(dst[:, h:n], src[:, h:n], AF.Copy)


def build_nc(reps=1, phases="full"):
    nc = bass.Bass("TRN2", debug=False, num_devices=B)

    x_t = nc.dram_tensor("x", [C, N], F32, kind="ExternalInput")
    wq_t = nc.dram_tensor("w_qkv", [C, 3 * C], F32, kind="ExternalInput")
    wp_t = nc.dram_tensor("w_proj", [C, C], F32, kind="ExternalInput")
    bp_t = nc.dram_tensor("b_proj", [C, 1], F32, kind="ExternalInput")
    y_t = nc.dram_tensor("y", [C, N], F32, kind="ExternalOutput")
    id_t = nc.inline_tensor(np.eye(128, dtype=np.float32), name="id128")

    xd, wqd, wpd, bpd, yd = x_t.ap(), wq_t.ap(), wp_t.ap(), bp_t.ap(), y_t.ap()

    with tile.TileContext(nc) as tc:
        with (
            tc.tile_pool(name="const", bufs=1) as cpool,
            tc.tile_pool(name="ys", bufs=6) as ypool,
            tc.tile_pool(name="ps", bufs=1, space="PSUM") as pspool,
        ):
            bank = [pspool.tile([128, C], F32, name=f"bank{i}", tag=f"bank{i}")
                    for i in range(8)]
            S = _Sched(nc, cpool, ypool, bank)

            # ---------------- persistent tiles -------------------------
            # double-buffered resident x: 2 x [4 chunks][8 slices] of
            # [128, 512] f32r  (2 x 8MB)
            x_sb = [
                [[cpool.tile([128, SL], F32R, name=f"x{bf}_{k}_{ns}",
                             tag=f"x{bf}_{k}_{ns}")
                  for ns in range(NS)] for k in range(KC)]
                for bf in range(2)
            ]
            id_sb = cpool.tile([128, 128], F32R, tag="id")
            # k/v sections of w_qkv: k in cols [0:512], v in [512:1024]
            wkv_sb = [cpool.tile([128, 2 * C], F32R, name=f"wkv{k}",
                                 tag=f"wkv{k}") for k in range(KC)]
            wp_sb = [cpool.tile([128, C], F32R, name=f"wp{k}", tag=f"wp{k}")
                     for k in range(KC)]
            bp_sb = [cpool.tile([128, 1], F32, name=f"bp{k}", tag=f"bp{k}")
                     for k in range(KC)]
            wqT_sb = [cpool.tile([128, C], F32R, name=f"wqT{k}", tag=f"wqT{k}")
                      for k in range(KC)]
            G_sb = [cpool.tile([128, C], F32R, name=f"G{k}", tag=f"G{k}")
                    for k in range(KC)]
            T_sb = [cpool.tile([128, C], F32R, name=f"T{k}", tag=f"T{k}")
                    for k in range(KC)]
            # M^T reuses T's storage: T is dead after the lg stage, and
            # Tile's WAR tracking orders the mt copies behind lg's reads.
            mt_sb = T_sb
            wy_sb = [cpool.tile([128, C], F32R, name=f"wy{k}", tag=f"wy{k}")
                     for k in range(KC)]
            xt_sb = [cpool.tile([128, C], F32R, name=f"xt{i}", tag=f"xt{i}")
                     for i in range(2)]
            bd = [cpool.tile([128, 128], F32, name=f"bd{p}", tag=f"bd{p}")
                  for p in range(HP)]
            bd2 = [cpool.tile([128, 128], F32R, name=f"bd2{p}", tag=f"bd2{p}")
                   for p in range(HP)]
            mx = cpool.tile([128, HP], F32, tag="mx")
            sbias = cpool.tile([128, HP], F32, tag="sbias")
            ssum = cpool.tile([128, HP], F32, tag="ssum")
            recip = cpool.tile([128, HP], F32, tag="recip")

            # ---------------- prologue DMAs ----------------------------
            nc.scalar.dma_start(id_sb[:], _r(id_t.ap()[:, :]))
            _emit_x_loads(nc, x_sb[0], xd)
            # weights: behind x(0) on the rings; first needed ~25us in
            for k in range(KC):
                r = slice(k * 128, (k + 1) * 128)
                eng = nc.sync if k % 2 == 0 else nc.scalar
                eng.dma_start(wkv_sb[k][:, 0:C], _r(wqd[r, C:2 * C]))
                eng.dma_start(wkv_sb[k][:, C:2 * C], _r(wqd[r, 2 * C:3 * C]))
            # q section into recycled ypool tiles (freed for phase C use)
            wq_q = []
            for k in range(KC):
                r = slice(k * 128, (k + 1) * 128)
                qt = ypool.tile([128, C], F32, tag="y_sb")
                eng = nc.sync if k % 2 == 0 else nc.scalar
                eng.dma_start(_r(qt[:]), _r(wqd[r, 0:C]))
                wq_q.append(qt)
            for k in range(KC):
                r = slice(k * 128, (k + 1) * 128)
                eng = nc.sync if k % 2 == 0 else nc.scalar
                eng.dma_start(wp_sb[k][:], _r(wpd[r, :]))
                eng.dma_start(bp_sb[k][:], bpd[r, :])
            # exp writes only the diagonal blocks of bd; the off-diagonal
            # zeros propagate into bd2 via the full-tile rowsum scaling
            for p in range(HP):
                nc.gpsimd.memset(bd[p][:], 0.0)

            # ---------------- prologue compute -------------------------
            # rep 0 phase A, standalone but software-pipelined
            for u in _a_units(S, x_sb[0], id_sb, xt_sb, G_sb):
                u()
            # Wq^T build (data-independent, once): staging banks 4,5
            for kq in range(KC):
                tp = bank[4 + kq % 2]
                for kci in range(KC):
                    nc.tensor.matmul(
                        _r(tp[:, kci * 128:(kci + 1) * 128]),
                        _r(wq_q[kci][:, kq * 128:(kq + 1) * 128]),
                        id_sb[:], is_transpose=True,
                        start=(kci == 0), stop=(kci == KC - 1),
                    )
                S.copy2(wqT_sb[kq], tp)

            # ---------------- steady-state rep loop --------------------
            for r in range(reps):
                nxt = (r + 1) % 2
                slots = []
                if r + 1 < reps:
                    _emit_x_loads(nc, x_sb[nxt], xd)
                    slots = _a_units(S, x_sb[nxt], id_sb, xt_sb, G_sb)
                slots = list(slots)
                si = 0

                def pull(n=1):
                    nonlocal si
                    for _ in range(n):
                        if si < len(slots):
                            slots[si]()
                            si += 1

                _emit_bc(nc, S, pull, wkv_sb, wp_sb, bp_sb, wqT_sb, G_sb,
                         T_sb, mt_sb, wy_sb, bd, bd2, mx, sbias, ssum, recip,
                         x_sb[r % 2], yd)
                pull(len(slots))  # drain any leftovers

    _split_multi_waits(nc)
    return nc


def _emit_x_loads(nc, xbuf, xd):
    """Load one full x image into an SBUF buffer, alternating rings."""
    for ns in range(NS):
        nsl = slice(ns * SL, (ns + 1) * SL)
        for k in range(KC):
            eng = nc.sync if (ns * KC + k) % 2 == 0 else nc.scalar
            eng.dma_start(xbuf[k][ns][:], _r(xd[k * 128:(k + 1) * 128, nsl]))


# G is symmetric: row-block 1 accumulates only cols [128:512], block 2
# only [256:512] (blocks 0 and 3 full — a 128-free f32r matmul costs the
# same 512 cycles as a 512-free one, so block 3 gains nothing from
# triangling). The 3 missing lower blocks are PE-transposed afterwards.
_GRAM_LO = [0, 128, 256, 0]


def _a_units(S, xbuf, id_sb, xt_sb, G_sb):
    """34 closures: unit i = [transposes of tile i][gram of tile i-1].
    Unit 32 = last gram + G PSUM->SBUF copies; unit 33 = the 3
    symmetric-fill transposes of G's lower blocks."""
    nc = S.nc
    units = []

    def mk(i):
        def unit():
            if i < NT:
                ns, t = divmod(i, TT)
                tsl = slice(t * 128, (t + 1) * 128)
                tp = S.bank[4 + i % 2]
                for k in range(KC):
                    nc.tensor.matmul(
                        _r(tp[:, k * 128:(k + 1) * 128]),
                        xbuf[k][ns][:, tsl], id_sb[:], is_transpose=True,
                        start=(k == 0), stop=(k == KC - 1),
                    )
                S.copy2(xt_sb[i % 2], tp)
            if 1 <= i <= NT:
                j = i - 1
                xt = xt_sb[j % 2]
                for k in range(KC):
                    lo = _GRAM_LO[k]
                    # accumulate G cols [lo:C] at bank offset 0 — an
                    # accumulating matmul dest off the bank base wedges
                    # the exec unit
                    nc.tensor.matmul(
                        S.bank[k][:, 0:C - lo], xt[:, k * 128:(k + 1) * 128],
                        xt[:, lo:C],
                        start=(j == 0), stop=(j == NT - 1),
                    )
            if i == NT:
                for k in range(KC):
                    lo = _GRAM_LO[k]
                    S.copy2(G_sb[k][:, lo:C], S.bank[k][:, 0:C - lo], n=C - lo)
            if i == NT + 1:
                # lower-block fills: (1,0)=(0,1)^T, (2,0)=(0,2)^T,
                # (2,1)=(1,2)^T — via PSUM staging on the B/C rotation
                fills = [(G_sb[0][:, 128:256], G_sb[1][:, 0:128], 0),
                         (G_sb[0][:, 256:384], G_sb[2][:, 0:128], 1),
                         (G_sb[1][:, 256:384], G_sb[2][:, 128:256], 0)]
                for fi, (src, dst, eng) in enumerate(fills):
                    tp = S.rot()
                    nc.tensor.matmul(_r(tp[:, 0:128]), src, id_sb[:],
                                     is_transpose=True, start=True, stop=True)
                    if eng == 0:
                        nc.vector.tensor_copy(dst, tp[:, 0:128])
                    else:
                        nc.scalar.activation(dst, tp[:, 0:128], AF.Copy)
        return unit

    for i in range(NT + 2):
        units.append(mk(i))
    return units


def _emit_bc(nc, S, pull, wkv_sb, wp_sb, bp_sb, wqT_sb, G_sb, T_sb, mt_sb,
             wy_sb, bd, bd2, mx, sbias, ssum, recip, xbuf, yd):
    # ---------------- phase B ------------------------------------------
    # T = G @ Wv   (v section = wkv cols [C:2C]); kc descending so the
    # symmetric-fill blocks (needed by kc=1,0) have time to land
    for kc in (3, 2, 1, 0):
        Tp = S.rot()
        for k2 in range(KC):
            nc.tensor.matmul(
                Tp[:], G_sb[k2][:, kc * 128:(kc + 1) * 128],
                wkv_sb[k2][:, C:2 * C],
                start=(k2 == 0), stop=(k2 == KC - 1),
            )
        S.copy2(T_sb[kc], Tp)

    # lg pairs: [128, 512] at full f32r rate; only each head's own
    # 64-col diagonal block is meaningful (rest junk, never read)
    lgp = []
    for p in range(HP):
        Lp = S.rot()
        lgp.append(Lp)
        for kc in range(KC):
            nc.tensor.matmul(
                Lp[:], wkv_sb[kc][:, p * 128:(p + 1) * 128], T_sb[kc][:],
                start=(kc == 0), stop=(kc == KC - 1),
            )
        if p == 1 or p == 3:
            pull()
        # softmax for this pair (reads PSUM directly)
        for par in range(2):
            psl = slice(64 * par, 64 * par + 64)
            csl = slice((2 * p + par) * 64, (2 * p + par) * 64 + 64)
            nc.vector.reduce_max(mx[psl, p:p + 1], Lp[psl, csl], axis=AX)
        nc.vector.tensor_scalar_mul(sbias[:, p:p + 1], mx[:, p:p + 1], -SCALE)
        for par in range(2):
            psl = slice(64 * par, 64 * par + 64)
            csl = slice((2 * p + par) * 64, (2 * p + par) * 64 + 64)
            nc.scalar.activation(
                bd[p][psl, psl], Lp[psl, csl], AF.Exp,
                bias=sbias[psl, p:p + 1], scale=SCALE,
            )
            nc.vector.reduce_sum(ssum[psl, p:p + 1], bd[p][psl, psl], axis=AX)
        nc.vector.reciprocal(recip[:, p:p + 1], ssum[:, p:p + 1])
        # fold 1/rowsum into the tiny exp matrix
        nc.vector.tensor_scalar_mul(bd2[p][:], bd[p][:], recip[:, p:p + 1])

    # M^T per pair, then Wy ci-sequential over 2 rotating banks
    for p in range(HP):
        mp = S.rot()
        nc.tensor.matmul(mp[:], bd2[p][:], wp_sb[p][:],
                         start=True, stop=True)
        S.copy2(mt_sb[p], mp)
        if p == 1 or p == 3:
            pull()
    for ci in range(KC):
        wyb = S.rot()
        for p in range(HP):
            nc.tensor.matmul(
                wyb[:], wqT_sb[p][:, ci * 128:(ci + 1) * 128], mt_sb[p][:],
                start=(p == 0), stop=(p == HP - 1),
            )
        S.copy2(wy_sb[ci], wyb)
        if ci == 1 or ci == 3:
            pull()

    # ---------------- phase C: y^T = Wy^T x + b ------------------------
    for ns in range(NS):
        nsl = slice(ns * SL, (ns + 1) * SL)
        for co in range(KC):
            yp = S.rot()
            for ci in range(KC):
                nc.tensor.matmul(
                    yp[:], wy_sb[ci][:, co * 128:(co + 1) * 128],
                    xbuf[ci][ns][:],
                    start=(ci == 0), stop=(ci == KC - 1),
                )
            ysb = S.ypool.tile([128, SL], F32, tag="y_sb")
            g = ns * KC + co
            if g % 2 == 0:
                nc.scalar.activation(
                    ysb[:], yp[:], AF.Identity,
                    bias=bp_sb[co][:, 0:1], scale=1.0,
                )
            else:
                nc.vector.tensor_scalar_add(ysb[:], yp[:], bp_sb[co][:, 0:1])
            eng = nc.sync if g % 2 == 0 else nc.scalar
            eng.dma_start(yd[co * 128:(co + 1) * 128, nsl], ysb[:])
            pull()


_NC_CACHE = None


def kernel(x, w_qkv, w_proj, b_proj, num_heads):
    x = np.ascontiguousarray(np.asarray(x, dtype=np.float32))
    w_qkv = np.ascontiguousarray(np.asarray(w_qkv, dtype=np.float32))
    w_proj = np.ascontiguousarray(np.asarray(w_proj, dtype=np.float32))
    b_proj = np.ascontiguousarray(np.asarray(b_proj, dtype=np.float32))
    assert int(num_heads) == NH
    assert x.shape == (B, C, H, W)

    xs = x.reshape(B, C, N)
    bp2 = b_proj.reshape(C, 1)
    in_maps = [
        {"x": xs[b], "w_qkv": w_qkv, "w_proj": w_proj, "b_proj": bp2}
        for b in range(B)
    ]
    global _NC_CACHE
    if _NC_CACHE is None:
        _NC_CACHE = build_nc()
    res = bass_utils.run_bass_kernel_spmd(_NC_CACHE, in_maps, list(range(B)))
    y = np.stack([res.results[b]["y"] for b in range(B)])
    return y.reshape(B, C, H, W).astype(np.float32)


if __name__ == "__main__":
    nc = build_nc(reps=2)
    n_inst = sum(len(bb.instructions) for bb in nc.main_func.blocks)
    print(f"built OK, {n_inst} instructions")
